# revision 3
# baseline (speedup 1.0000x reference)
"""BertPooler segment-reduce, k-sliced ReduceScatter layout.

Core c owns hidden-dim slice c*128..(c+1)*128 of ALL 64 batches:
  phase 1: gather every window's tokens restricted to the k-slice
    (packed slots of 4 consecutive h_k rows = 1KB contiguous reads),
    masked matmuls -> featsT k-block [128, 64 cls + 128 windows] f32.
  phase 2: partial pooler BEFORE the collective: for each output
    column block jc, opT_jc[128 j, 64 b] = sum_s Wslice^T @ ft + b/8,
    written to DRAM [1024, 64] f32.
  phase 3: ReduceScatter(add) sums the 8 partials and hands core c
    exactly its final output rows [128 j, 64 b] (32KB payload vs the
    384KB AllGather, whose ~40GB/s transport was the bottleneck);
    tanh, store.
"""

import numpy as np

B, S, H = 64, 512, 1024
N_CORES = 8
P = 128
KS = H // N_CORES           # 128-dim k-slice per core
NWG = 2 * B                 # 128 global windows (subj 0..63, obj 64..127)
TPP = 4
# mask cols: eye64 | per-block window masks (4 par x W_gb each)
M_EYE = 0
M_BLK = 64

_cache: dict = {}


def _pack_blocks(subj, obj):
    """Slot list packed so no window straddles a 128-slot block.

    Returns (slots, blocks): slots = [(w, off) or None(pad)], blocks =
    tuple of (w_lo, n_windows) per 128-slot block."""
    slots, blocks = [], []
    cur_lo, cur_w = 0, 0
    for w in range(NWG):
        rng = (subj, obj)[w // B]
        b = w % B
        ln = int(rng[b, 1] - rng[b, 0])
        k = (ln + TPP - 1) // TPP
        if len(slots) % P + k > P:            # pad to block boundary
            while len(slots) % P:
                slots.append(None)
        if len(slots) % P == 0 and slots:
            blocks.append((cur_lo, cur_w))
            cur_lo, cur_w = w, 0
        if not slots:
            cur_lo = w
        for off in range(k):
            slots.append((w, off))
        cur_w = w - cur_lo + 1
    while len(slots) % P:
        slots.append(None)
    blocks.append((cur_lo, cur_w))
    return slots, tuple(blocks)


def _build(blocks, reps=1, hw_loop=False, bufs=1):
    import contextlib
    import concourse.bass as bass
    import concourse.tile as tile
    from concourse import bacc, mybir

    f32 = mybir.dt.float32
    bf16 = mybir.dt.bfloat16
    i32 = mybir.dt.int32
    NBLK = len(blocks)
    bases = []
    base = M_BLK
    for (_, wg) in blocks:
        bases.append(base)
        base += 4 * wg
    MSKW = base

    nc = bacc.Bacc("TRN2", target_bir_lowering=False, debug=False,
                   num_devices=N_CORES)
    h = nc.dram_tensor("h", [B * S, KS], bf16, kind="ExternalInput")
    idxd = nc.dram_tensor("idx", [P, NBLK], i32, kind="ExternalInput")
    mskd = nc.dram_tensor("msk", [P, MSKW], bf16, kind="ExternalInput")
    biasd = nc.dram_tensor("biasd", [1, H + B], bf16, kind="ExternalInput")
    wpk = nc.dram_tensor("wpk", [P, 3 * H], bf16, kind="ExternalInput")
    out = nc.dram_tensor("out", [KS, B], bf16, kind="ExternalOutput")
    cc_ins = [nc.dram_tensor(f"cc_in{i}", [H, B], f32)
              for i in range(min(bufs, 4))]
    rs_outs = [nc.dram_tensor(f"rs_out{i}", [KS, B], f32)
               for i in range(min(bufs, 4))]

    with tile.TileContext(nc) as tc:
        with (
            tc.tile_pool(name="work", bufs=bufs) as wpool,
            tc.tile_pool(name="psum", bufs=min(bufs, 2),
                         space="PSUM") as ppool,
        ):
          loop_ctx = (tc.For_i(0, reps, 1) if hw_loop
                      else contextlib.nullcontext())
          with loop_ctx:
            for _rep in range(1 if hw_loop else reps):
                cc_in = cc_ins[_rep % len(cc_ins)]
                rs_out = rs_outs[_rep % len(rs_outs)]
                idx_t = wpool.tile([P, NBLK], i32, tag="idx")
                nc.gpsimd.dma_start(idx_t[:], idxd[:, :])
                msk_t = wpool.tile([P, MSKW], bf16, tag="msk")
                nc.sync.dma_start(msk_t[:], mskd[:, :])
                bias_t = wpool.tile([1, H + B], bf16, tag="bias")
                nc.sync.dma_start(bias_t[:], biasd[:, :])
                cls_t = wpool.tile([B, KS], bf16, tag="cls")
                h_bsd = h.ap().rearrange("(b s) k -> b s k", s=S)
                nc.scalar.dma_start(cls_t[:], h_bsd[:, 0, :])

                gts, gds = [], []
                for gb in range(NBLK):
                    gt = wpool.tile([P, TPP * KS], bf16, tag=f"gt{gb}")
                    gd = nc.gpsimd.indirect_dma_start(
                        out=gt[:, :],
                        out_offset=None,
                        in_=h.ap(),
                        in_offset=bass.IndirectOffsetOnAxis(
                            ap=idx_t[:, gb:gb + 1], axis=0),
                        bounds_check=B * S - 1,
                        oob_is_err=False,
                    )
                    gts.append(gt)
                    gds.append(gd)

                w_t = wpool.tile([P, 3 * H], bf16, tag="wt")
                HW2 = 3 * H // 2
                for i, eng in enumerate((nc.scalar, nc.sync)):
                    wd = eng.dma_start(w_t[:, i * HW2:(i + 1) * HW2],
                                       wpk[:, i * HW2:(i + 1) * HW2])
                    bass._add_dep_helper(wd.ins, gds[-1].ins, sync=True,
                                         reason="stagger W after gathers")

                # featsT k-block [128, 64 cls | 128 windows] f32
                fp = ppool.tile([P, 64 + NWG], f32, tag="fp",
                                space="PSUM")
                nc.tensor.matmul(out=fp[:, 0:64],
                                 lhsT=cls_t[0:B, :],
                                 rhs=msk_t[0:B, M_EYE:M_EYE + 64],
                                 start=True, stop=True)
                for gb, (w_lo, wg) in enumerate(blocks):
                    for par in range(TPP):
                        nc.tensor.matmul(
                            out=fp[:, 64 + w_lo:64 + w_lo + wg],
                            lhsT=gts[gb][:, par * KS:(par + 1) * KS],
                            rhs=msk_t[:, bases[gb] + par * wg:
                                      bases[gb] + (par + 1) * wg],
                            start=(par == 0), stop=(par == TPP - 1))
                ft_sb = wpool.tile([P, 64 + NWG], bf16, tag="ftsb")
                nc.vector.tensor_copy(ft_sb[:], fp[:])

                # partial pooler (pre-collective): 8 col-blocks of 128
                # fused into ONE PSUM bank [128, 8*64] (PSUM tiles are
                # bank-granular; separate tiles would need 9 banks)
                op_all = ppool.tile([P, N_CORES * B], f32, tag="opall",
                                    space="PSUM")
                for jc in range(N_CORES):
                    oc = op_all[:, jc * B:(jc + 1) * B]
                    nc.tensor.matmul(
                        out=oc,
                        lhsT=bias_t[0:1, jc * P:(jc + 1) * P],
                        rhs=bias_t[0:1, H:H + B],
                        start=True, stop=False)
                    for s in range(3):
                        nc.tensor.matmul(
                            out=oc,
                            lhsT=w_t[:, s * H + jc * P:s * H + (jc + 1) * P],
                            rhs=ft_sb[:, _ft_off(s):_ft_off(s) + 64],
                            start=False, stop=(s == 2))
                ps_all = wpool.tile([P, N_CORES * B], f32, tag="psall")
                nc.vector.tensor_copy(ps_all[:], op_all[:])
                nc.sync.dma_start(
                    cc_in.ap().rearrange("(jc p) b -> p jc b", p=P),
                    ps_all[:])

                nc.gpsimd.collective_compute(
                    "ReduceScatter", mybir.AluOpType.add,
                    replica_groups=[list(range(N_CORES))],
                    ins=[cc_in.ap().opt()],
                    outs=[rs_out.ap().opt()],
                )
                rsb = wpool.tile([KS, B], f32, tag="rsb")
                nc.scalar.dma_start(rsb[:], rs_out.ap()[:, :])
                o_sb = wpool.tile([KS, B], bf16, tag="osb")
                nc.scalar.activation(
                    out=o_sb[:KS, :], in_=rsb[:KS, :],
                    func=mybir.ActivationFunctionType.Tanh)
                nc.scalar.dma_start(out[:, :], o_sb[:])

    nc.compile()
    return nc


def _ft_off(s):
    """feats col offset of segment s: cls 0..64, subj 64..128, obj 128..192."""
    return s * 64


def _host_inputs(hidden_states, subj, obj, wt_full, bias_full, slots,
                 blocks, c):
    import ml_dtypes
    NBLK = len(blocks)
    bases = []
    base = M_BLK
    for (_, wg) in blocks:
        bases.append(base)
        base += 4 * wg
    MSKW = base

    key_m = ("mskidx",)
    if key_m not in _cache:
        idx = np.zeros((P, NBLK), np.int32)
        msk = np.zeros((P, MSKW), ml_dtypes.bfloat16)
        msk[0:B, M_EYE:M_EYE + B] = np.eye(B, dtype=np.float32)
        ranges = (subj, obj)
        for i, sl in enumerate(slots):
            gb, p = divmod(i, P)
            if sl is None:
                continue
            w, off = sl
            rng = ranges[w // B]
            b = w % B
            start = int(rng[b, 0])
            ln = int(rng[b, 1] - rng[b, 0])
            idx[p, gb] = min(b * S + start + TPP * off, B * S - TPP)
            w_lo, wg = blocks[gb]
            for par in range(TPP):
                j = TPP * off + par
                if j < ln:
                    msk[p, bases[gb] + par * wg + (w - w_lo)] = (
                        np.float32(1.0 / ln))
        _cache[key_m] = (idx, msk)
    idx, msk = _cache[key_m]

    key_h = ("hk", c)
    if key_h not in _cache:
        _cache[key_h] = np.ascontiguousarray(
            hidden_states[:, :, c * KS:(c + 1) * KS].reshape(B * S, KS)
        ).astype(ml_dtypes.bfloat16)

    key_w = ("wpk", c)
    if key_w not in _cache:
        wt = np.asarray(wt_full)    # [3072, 1024]
        _cache[key_w] = np.ascontiguousarray(np.concatenate(
            [wt[s * H + c * KS:s * H + (c + 1) * KS, :] for s in range(3)],
            axis=1)).astype(ml_dtypes.bfloat16)   # [128, 3*1024]

    biasd = np.zeros((1, H + B), ml_dtypes.bfloat16)
    biasd[0, 0:H] = (np.asarray(bias_full).reshape(-1) / N_CORES
                     ).astype(np.float32)
    biasd[0, H:H + B] = 1.0
    return {
        "h": _cache[key_h],
        "idx": idx,
        "msk": msk,
        "biasd": biasd,
        "wpk": _cache[key_w],
    }


def kernel(hidden_states, subj_range, obj_range, W, b):
    from concourse.bass_utils import run_bass_kernel_spmd

    hidden_states = np.asarray(hidden_states, dtype=np.float32)
    subj = np.asarray(subj_range).astype(np.int64)
    obj = np.asarray(obj_range).astype(np.int64)
    W = np.asarray(W, dtype=np.float32)
    b = np.asarray(b, dtype=np.float32)

    for k in [k for k in _cache if isinstance(k, tuple) and k[0] != "nc"]:
        del _cache[k]
    slots, blocks = _pack_blocks(subj, obj)
    key = ("nc", blocks)
    if key not in _cache:
        _cache[key] = _build(blocks)
    nc = _cache[key]
    wt_full = np.ascontiguousarray(W.T)
    bias_full = np.ascontiguousarray(b[None, :])
    in_maps = [_host_inputs(hidden_states, subj, obj, wt_full, bias_full,
                            slots, blocks, c) for c in range(N_CORES)]
    res = run_bass_kernel_spmd(nc, in_maps, core_ids=list(range(N_CORES)))
    out = np.empty((B, H), np.float32)
    for c in range(N_CORES):
        out[:, c * KS:(c + 1) * KS] = res.results[c]["out"].astype(
            np.float32).T
    return out
